# revision 27
# baseline (speedup 1.0000x reference)
"""Trainium2 Bass kernel for nn_Align: batched quaternion->rotmat + rigid transform.

reference math (per structure j of 64):
    q = (1, b, c, d) / sqrt(s),  s = 1 + b^2 + c^2 + d^2
    R = rotmat(q)                       # 3x3
    out[j] = pred[j] @ R + t[j]         # [91,3] @ [3,3] + [3]

Sharding: data-parallel over the 8 NeuronCores, 8 structures per core.

Per-core layout: partitions = (structure j:8, out-channel n:3, point-group
g:5) = 120, with 19 points per group (91 points padded to 95; the pad is
sliced off host-side).  Every rotation-matrix entry has the form
    R[k,n] = (x*y + z) * (2/s) - w
with (x,y,z,w) in {1,b,c,d,0,+-...}, so column n of R is computed per
partition from host-packed operand vectors X,Y,Z,W via
    C = (X*Y + Z) * (2/s) - W            # C = (R[0,n], R[1,n], R[2,n])
where one fused scalar_tensor_tensor produces BOTH the x*y column products
and (via a sign-compensated accum_out: the packed +-x*y slots cancel) s/2
in a single DVE op.  The transform is then only 3 multiply-add ops over
the 19-point free dim:
    out_n = ((x0*C0 + t_n) + x1*C1) + x2*C2
for a 7-op DVE chain total.  Output is written channel-planar ([8,3,95]
DRAM) so each partition's row is contiguous (120 single-descriptor rows);
the host interleaves channels during the unshard gather.

Inputs ride TWO xbar transpose DMAs, packed host-side as uint16 views
transposed to [rows, 128] and read back through exact f32 bitcast APs:
the chain-gating params tile (4 16x128 tiles) is issued in SP's first
slot before the preamble barrier, while the coordinate tile (8 tiles,
not needed until the 5th op) is issued on the Activation queue after
Act's barrier so its descriptor generation never gates the preamble
release.  dma_start_transpose descriptor generation is far cheaper than
a strided copy's.  The SP barrier's per-engine DGE drain retires the
params DMA's completion bookkeeping as early as the hardware allows
(Act's Block-exit drain does the same for the coordinate tile), and the
Block exit barrier's drains hold the kernel end until the output DMA has
fully completed.
"""

import numpy as np

NCORES = 8
J = 8          # structures per core
N = 3          # output channels (partition dim)
G = 5          # point groups per (structure, channel)
Q = 19         # points per group (G*Q = 95 >= 91)
NPTS = 91
PADPTS = G * Q  # 95
PARTS = J * N * G  # 120 used partitions (SBUF tile is 128)

# Two per-partition f32 rows, each landed by its own xbar transpose DMA so
# the small params tile (4 transpose tiles) gates the compute chain instead
# of the full payload (11 tiles):
#   params row (32 f32 = 64 u16 rows): [A(10): 1 b c d x0 x1 x2 x0 x1 x2 |
#     B(10): .5 .5b .5c .5d y0 y1 y2 -y0 -y1 -y2 | Z(3) | W(3) | t(1) | pad(5)]
#   coords row (64 f32 = 128 u16 rows): [X0(19) | X1(19) | X2(19) | pad(7)]
# One fused (A*B, accum) op yields the column products AND s/2: the +-x*y
# accumulator slots cancel, leaving 0.5*(1+b^2+c^2+d^2).
PA, PBv, PZ, PW, PT = 0, 10, 20, 23, 26
PLEN = 32                # params f32 row length (27 used + 5 pad)
PULEN = 2 * PLEN         # 64 uint16 rows (= 4 xbar tiles)
XLEN = 64                # coords f32 row length (57 used + 7 pad)
XULEN = 2 * XLEN         # 128 uint16 rows (= 8 xbar tiles)

_cache = {}


def _build_nc():
    import concourse.bass as bass
    import concourse.mybir as mybir

    f32 = mybir.dt.float32
    u16 = mybir.dt.uint16
    Alu = mybir.AluOpType

    nc = bass.Bass()
    # host packs pu/xu[e, p] = uint16-view(row[p])[e]; the xbar transpose
    # DMAs restore [p, e] in SBUF, bit-identical to the f32 rows
    pu_d = nc.dram_tensor("pu", [PULEN, 128], u16, kind="ExternalInput")
    xu_d = nc.dram_tensor("xu", [XULEN, 128], u16, kind="ExternalInput")
    # channel-planar output: [j, n, 95]; host transposes/slices to [j,91,n]
    o3 = nc.dram_tensor("o3", [J, N, PADPTS], f32, kind="ExternalOutput")

    with (
        nc.sbuf_tensor([128, PULEN], u16) as PU_t,
        nc.sbuf_tensor([128, XULEN], u16) as XU_t,
        nc.sbuf_tensor([PARTS, 10], f32) as PR_t,
        nc.sbuf_tensor([PARTS, 1], f32) as S2_t,
        nc.sbuf_tensor([PARTS, 1], f32) as INV_t,
        nc.sbuf_tensor([PARTS, 3], f32) as NU_t,
        nc.sbuf_tensor([PARTS, 3], f32) as C_t,
        nc.sbuf_tensor([PARTS, Q], f32) as A1_t,
        nc.sbuf_tensor([PARTS, Q], f32) as A2_t,
        nc.sbuf_tensor([PARTS, Q], f32) as O_t,
        nc.semaphore("p_sem") as p_sem,
        nc.semaphore("x_sem") as x_sem,
        nc.semaphore("v_sem") as v_sem,
        nc.semaphore("o_sem") as o_sem,
        nc.Block() as block,
    ):
        def pslice(a, b):
            # f32 view [PARTS, b-a] over the uint16-typed params rows
            return PU_t[0:PARTS, 2 * a:2 * b].bitcast(f32)

        def xslice(a, b):
            # f32 view [PARTS, b-a] over the uint16-typed coord rows
            return XU_t[0:PARTS, 2 * a:2 * b].bitcast(f32)

        A1 = A1_t[:, :]
        A2 = A2_t[:, :]
        O = O_t[:, :]

        def _pseudo_barrier(eng):
            # NRT expands this to a per-engine DGE drain + all-engine
            # barrier on runtime semaphores outside the kernel sem range --
            # stale-state proof, and the drain retires each engine's
            # outstanding DMA state.
            eng.isa(
                nc.isa.Opcode.NEURON_ISA_TPB_OPCODE_PSEUDO_SYNC_BARRIER,
                {},
                struct_name="NEURON_ISA_TPB_UNKNOWN_STRUCT",
                verify=False,
            )

        @block.gpsimd
        def _(gpsimd):
            # Stale-semaphore preamble: semaphores are NOT reset between NEFF
            # executions, and waits here use absolute values.  Clear every
            # sem this kernel waits on or increments, THEN barrier -- without
            # the barrier an engine can pass its first wait on a stale value
            # before the clear lands (observed as a HW deadlock).
            #
            # The input DMAs are issued by SP/Act BEFORE this preamble
            # completes, so p_sem/x_sem are excluded from the dma_reset
            # (draining them would nuke the in-flight descriptors; their
            # queues are safe because the previous run of this kernel fully
            # retired both input DMAs before its exit barrier).  The
            # clear-vs-inc race is safe by construction: a DGE completion
            # inc lands >=1.5us after issue (descriptor gen + DGE delay +
            # transfer + sem prop), while these clears retire within ~400ns
            # of kernel start; a pathologically late clear would zero the
            # sem after its inc and the consumer wait would hang (fail-stop,
            # not silent corruption).  All consumer waits still execute
            # after the barrier, hence after the clears.
            assert x_sem.num == p_sem.num + 1
            assert v_sem.num == x_sem.num + 1 and o_sem.num == v_sem.num + 1
            gpsimd.dma_reset(range(v_sem.num, o_sem.num + 1))
            gpsimd.sem_clear(range(p_sem.num, o_sem.num + 1))
            _pseudo_barrier(gpsimd)

        @block.tensor
        def _(tensor):
            _pseudo_barrier(tensor)

        @block.scalar
        def _(scalar):
            # coord planes ride the Activation queue, parallel with the
            # chain-gating params tile on SP; not needed until the 5th op.
            # Issued AFTER Act's barrier so the preamble release is not
            # gated by this DMA's descriptor generation (the completion
            # bookkeeping is retired by Act's Block-exit drain instead),
            # and so the x_sem increment cannot race the preamble clears
            # at all.
            _pseudo_barrier(scalar)
            scalar.dma_start_transpose(
                out=XU_t[:, :], in_=xu_d[:, :]
            ).then_inc(x_sem, 16)

        @block.sync
        def _(sync):
            sync.dma_start_transpose(
                out=PU_t[:, :], in_=pu_d[:, :]
            ).then_inc(p_sem, 16)
            _pseudo_barrier(sync)
            sync.wait_ge(v_sem, 7)
            sync.dma_start(
                out=o3[:, :, :].rearrange("j n (g q) -> (j n g) q", g=G),
                in_=O,
            ).then_inc(o_sem, 16)
            sync.wait_ge(o_sem, 16)

        @block.vector
        def _(vector):
            _pseudo_barrier(vector)
            vector.wait_ge(p_sem, 16)

            # DVE streaming RAW is not safe without sem sync (HW-verified):
            # every op bumps v_sem; consumers wait on the cumulative count.
            def op(k, *args, **kw):
                return getattr(vector, k)(*args, **kw).then_inc(v_sem, 1)

            # ---- rotation column C = (X*Y + Z) * (2/s) - W ----
            # one fused op: PR = A*B elementwise (PR[4:7] = x*y column
            # products) and accum = sum(PR) = s/2 (the +-x*y slots cancel).
            # (tensor_tensor_reduce hits "ISA wrong length" in neuronxcc
            # codegen; scalar_tensor_tensor's accum_out compiles fine)
            op("scalar_tensor_tensor", out=PR_t[:, :],                   # 1
               in0=pslice(PA, PA + 10), scalar=1.0,
               in1=pslice(PBv, PBv + 10),
               op0=Alu.mult, op1=Alu.mult, accum_out=S2_t[:, :])
            vector.wait_ge(v_sem, 1)
            op("reciprocal", out=INV_t[:, :], in_=S2_t[:, :])            # 2  2/s
            op("tensor_tensor", out=NU_t[:, :], in0=PR_t[:, 4:7],        # 3
               in1=pslice(PZ, PZ + 3), op=Alu.add)
            vector.wait_ge(v_sem, 3)
            op("scalar_tensor_tensor", out=C_t[:, :], in0=NU_t[:, :],    # 4
               scalar=INV_t[:, 0:1], in1=pslice(PW, PW + 3),
               op0=Alu.mult, op1=Alu.subtract)

            # ---- transform: out_n = ((x0*C0 + t) + x1*C1) + x2*C2 ----
            vector.wait_ge(x_sem, 16)
            vector.wait_ge(v_sem, 4)
            op("tensor_scalar", out=A1, in0=xslice(0, Q),          # 5
               scalar1=C_t[:, 0:1], scalar2=pslice(PT, PT + 1),
               op0=Alu.mult, op1=Alu.add)
            vector.wait_ge(v_sem, 5)
            op("scalar_tensor_tensor", out=A2,                           # 6
               in0=xslice(Q, 2 * Q),
               scalar=C_t[:, 1:2], in1=A1, op0=Alu.mult, op1=Alu.add)
            vector.wait_ge(v_sem, 6)
            op("scalar_tensor_tensor", out=O,                            # 7
               in0=xslice(2 * Q, 3 * Q),
               scalar=C_t[:, 2:3], in1=A2, op0=Alu.mult, op1=Alu.add)

    return nc


def get_nc():
    if "nc" not in _cache:
        _cache["nc"] = _build_nc()
    return _cache["nc"]


def shard_inputs(pred_coor, r_vector, t_vector):
    n_total = pred_coor.shape[0]
    b, c, d = r_vector[:, 0], r_vector[:, 1], r_vector[:, 2]
    one = np.ones_like(b)
    zero = np.zeros_like(b)

    # per-channel operand vectors: R[k,n] = (x*y+z)*(2/s) - w
    X = np.empty((n_total, N, 3), dtype=np.float32)
    Y = np.empty((n_total, N, 3), dtype=np.float32)
    Z = np.empty((n_total, N, 3), dtype=np.float32)
    W = np.empty((n_total, N, 3), dtype=np.float32)
    X[:, 0] = np.stack([b, b, b], -1)
    Y[:, 0] = np.stack([b, c, d], -1)
    Z[:, 0] = np.stack([one, d, -c], -1)
    W[:, 0] = np.stack([one, zero, zero], -1)
    X[:, 1] = np.stack([b, c, c], -1)
    Y[:, 1] = np.stack([c, c, d], -1)
    Z[:, 1] = np.stack([-d, one, b], -1)
    W[:, 1] = np.stack([zero, one, zero], -1)
    X[:, 2] = np.stack([b, c, d], -1)
    Y[:, 2] = np.stack([d, d, d], -1)
    Z[:, 2] = np.stack([c, -b, one], -1)
    W[:, 2] = np.stack([zero, zero, one], -1)

    b4 = np.stack([one, b, c, d], -1)[:, None, :]  # [n_total, 1, 4]
    pk = np.zeros((n_total, N, PLEN), dtype=np.float32)
    pk[:, :, PA:PA + 4] = b4
    pk[:, :, PA + 4:PA + 7] = X
    pk[:, :, PA + 7:PA + 10] = X
    pk[:, :, PBv:PBv + 4] = 0.5 * b4
    pk[:, :, PBv + 4:PBv + 7] = Y
    pk[:, :, PBv + 7:PBv + 10] = -Y
    pk[:, :, PZ:PZ + 3] = Z
    pk[:, :, PW:PW + 3] = W
    pk[:, :, PT] = t_vector
    # replicate params over point groups
    pk = np.broadcast_to(pk[:, :, None, :], (n_total, N, G, PLEN))
    pk = np.ascontiguousarray(pk).reshape(n_total * N * G, PLEN)

    # coords, planar per partition: [q=19 for m=0 | m=1 | m=2 | pad]
    padded = np.zeros((n_total, PADPTS, 3), dtype=np.float32)
    padded[:, :NPTS] = pred_coor
    # (j, g, q, m) -> (j, g, m, q)
    xt3 = padded.reshape(n_total, G, Q, 3).transpose(0, 1, 3, 2)
    xt = np.zeros((n_total, N, G, XLEN), dtype=np.float32)
    xt[:, :, :, 0:3 * Q] = xt3.reshape(n_total, 1, G, 3 * Q)
    xt = np.ascontiguousarray(xt).reshape(n_total * N * G, XLEN)

    jper = J * N * G
    shards = []
    for cc in range(NCORES):
        prow = np.zeros((128, PLEN), dtype=np.float32)
        prow[:jper] = pk[cc * jper:(cc + 1) * jper]
        xrow = np.zeros((128, XLEN), dtype=np.float32)
        xrow[:jper] = xt[cc * jper:(cc + 1) * jper]
        # uint16 views, transposed for the xbar transpose DMAs
        pu = np.ascontiguousarray(prow.view(np.uint16).reshape(128, PULEN).T)
        xu = np.ascontiguousarray(xrow.view(np.uint16).reshape(128, XULEN).T)
        shards.append({"pu": pu, "xu": xu})
    return shards


def run(pred_coor, r_vector, t_vector, trace=False):
    from concourse.bass_utils import run_bass_kernel_spmd

    nc = get_nc()
    in_maps = shard_inputs(pred_coor, r_vector, t_vector)
    res = run_bass_kernel_spmd(nc, in_maps, list(range(NCORES)), trace=trace)
    full = np.concatenate(
        [
            np.asarray(res.results[cc]["o3"]).transpose(0, 2, 1)[:, :NPTS, :]
            for cc in range(NCORES)
        ],
        axis=0,
    )
    return np.ascontiguousarray(full), res


def kernel(pred_coor, r_vector, t_vector):
    pred_coor = np.asarray(pred_coor, dtype=np.float32)
    r_vector = np.asarray(r_vector, dtype=np.float32)
    t_vector = np.asarray(t_vector, dtype=np.float32)
    full, _ = run(pred_coor, r_vector, t_vector, trace=False)
    return full
